# revision 40
# baseline (speedup 1.0000x reference)
"""Trainium2 Bass kernel for nn_Ag3ChargeStateModel (GNN message passing).

Strategy (8 NeuronCores, SPMD):
  - Shard atoms across cores: core r owns atoms [r*256, (r+1)*256), processed
    as 2 partition-tiles of 128 atoms. Positions replicated to every core.
  - d2[i,j] = |pi|^2 + |pj|^2 - 2 pi.pj via one PE matmul with a rank-5
    contraction: lhsT = [px,py,pz,sq,1] (K=5), rhs = [-2px,-2py,-2pz,1,sq].
  - Neighbor mask (0 < d2 < cutoff^2) folded into the distances: masked-out
    pairs get d2 := 1e8 so dist = 1e4 and every RBF underflows to exactly 0.
  - RBF expansion + neighbor sum fused into ONE scalar-engine op per center:
    Derivative_Erf(sqrt(g)*d - sqrt(g)*c_k) = (2/sqrt(pi))*exp(-g(d-c_k)^2),
    with accum_out performing the row (neighbor) reduction. The 2/sqrt(pi)
    factor is folded into W1.
  - MLP on PE: feat^T via PE transpose, h^T = Silu(W1f^T @ feat^T + b1'),
    e^T = W2^T @ h^T. The charge-state embedding contribution is constant
    across atoms, so emb @ W1[16:] is folded into the bias b1'.
  - Per-atom energies DMA'd out; host sums the 8 partial results (psum).
"""

import numpy as np

N_ATOMS = 2048
N_CORES = 8
ATOMS_PER_CORE = N_ATOMS // N_CORES  # 256
P = 128                              # partition tile
N_TILES = ATOMS_PER_CORE // P        # 2
N_RBF = 16
N_HIDDEN = 32
CUTOFF = 5.0
BIG_D2 = 1.0e8                       # masked pairs: dist=1e4 -> RBF arg ~3e4 -> 0
SQRT_BIAS = 4.0e-5                   # keeps the sqrt input positive under f32 cancellation noise

_CACHE = {}


def _rbf_constants():
    centers = np.linspace(0.0, np.float32(CUTOFF), N_RBF, dtype=np.float32)
    width = centers[1] - centers[0]
    gamma = np.float32(1.0) / (width * width)
    sqrtg = np.float32(np.sqrt(np.float64(gamma)))
    return centers, gamma, sqrtg


def _build_program(reps=1):
    from concourse import mybir, bacc
    import concourse.tile as tile

    AF = mybir.ActivationFunctionType
    ALU = mybir.AluOpType
    FP32 = mybir.dt.float32

    centers, gamma, sqrtg = _rbf_constants()

    nc = bacc.Bacc("TRN2", target_bir_lowering=False, debug=False)

    lhsT_d = nc.dram_tensor("lhsT", [5, ATOMS_PER_CORE], FP32, kind="ExternalInput").ap()
    rhs_d = nc.dram_tensor("rhs", [5, N_ATOMS], FP32, kind="ExternalInput").ap()
    w1f_d = nc.dram_tensor("w1f", [N_RBF, N_HIDDEN], FP32, kind="ExternalInput").ap()
    w2_d = nc.dram_tensor("w2", [N_HIDDEN, 1], FP32, kind="ExternalInput").ap()
    b1p_d = nc.dram_tensor("b1p", [N_HIDDEN, 1], FP32, kind="ExternalInput").ap()
    ident_d = nc.dram_tensor("ident", [P, P], FP32, kind="ExternalInput").ap()
    bident_d = nc.dram_tensor("bident", [P, P], FP32, kind="ExternalInput").ap()
    rbfb_d = nc.dram_tensor("rbfb", [P, N_RBF + 1], FP32, kind="ExternalInput").ap()
    eout_d = nc.dram_tensor("eout", [N_TILES, P], FP32, kind="ExternalOutput").ap()

    with tile.TileContext(nc) as tc:
        with (
            tc.tile_pool(name="const", bufs=1) as cpool,
            tc.tile_pool(name="work", bufs=2) as wpool,
            tc.tile_pool(name="mlp", bufs=1) as mpool,
            tc.tile_pool(name="psum_big", bufs=1, space="PSUM") as pbig,
            tc.tile_pool(name="psum_small", bufs=1, space="PSUM") as psmall,
        ):
            # ---- constant loads ----
            rhs_s = cpool.tile([5, N_ATOMS], FP32, tag="rhs")
            nc.sync.dma_start(rhs_s[:], rhs_d[:])
            lhsT_s = cpool.tile([5, ATOMS_PER_CORE], FP32, tag="lhsT")
            nc.sync.dma_start(lhsT_s[:], lhsT_d[:])
            w1f_s = cpool.tile([N_RBF, N_HIDDEN], FP32, tag="w1f")
            nc.sync.dma_start(w1f_s[:], w1f_d[:])
            w2_s = cpool.tile([N_HIDDEN, 1], FP32, tag="w2")
            nc.sync.dma_start(w2_s[:], w2_d[:])
            b1p_s = cpool.tile([N_HIDDEN, 1], FP32, tag="b1p")
            nc.sync.dma_start(b1p_s[:], b1p_d[:])
            ident_s = cpool.tile([P, P], FP32, tag="ident")
            nc.sync.dma_start(ident_s[:], ident_d[:])
            rbfb_s = cpool.tile([P, N_RBF + 1], FP32, tag="rbfb")
            nc.sync.dma_start(rbfb_s[:], rbfb_d[:])
            bident_s = cpool.tile([P, P], FP32, tag="bident")
            nc.sync.dma_start(bident_s[:], bident_d[:])

            def body():
                _emit_body(
                    nc, tc, wpool, mpool, pbig, psmall,
                    lhsT_s, rhs_s, w1f_s, w2_s, b1p_s, ident_s, rbfb_s, bident_s,
                    eout_d, sqrtg, AF, ALU, mybir, FP32,
                )

            if reps == 1:
                body()
            else:
                with tc.For_i(0, reps, 1, staggered_reset=True):
                    body()

    nc.compile()
    return nc


def _emit_body(
    nc, tc, wpool, mpool, pbig, psmall,
    lhsT_s, rhs_s, w1f_s, w2_s, b1p_s, ident_s, rbfb_s, bident_s,
    eout_d, sqrtg, AF, ALU, mybir, FP32,
):
            dist_tiles = []
            feat_tiles = []

            # ---- distances + masking (PE + DVE), per atom tile ----
            for t in range(N_TILES):
                d2_p = pbig.tile([P, N_ATOMS], FP32, tag="d2")
                for nb in range(N_ATOMS // 512):
                    nc.tensor.matmul(
                        d2_p[:, nb * 512:(nb + 1) * 512],
                        lhsT_s[:, t * P:(t + 1) * P],
                        rhs_s[:, nb * 512:(nb + 1) * 512],
                        start=True,
                        stop=(nb != 0),
                    )
                # rhs columns are rotated per core so this core's own atoms sit
                # at columns [0, 256): spike the self-pair diagonal with BIG*I
                # so it lands beyond the cutoff (exact diagonal exclusion).
                nc.tensor.matmul(
                    d2_p[:, t * P:(t + 1) * P],
                    bident_s[:],
                    ident_s[:],
                    start=False,
                    stop=True,
                )
                # cutoff mask: add BIG where d2 >= cutoff^2 (diagonal already
                # spiked beyond the cutoff by the identity matmul above).
                u_s = wpool.tile([P, N_ATOMS], FP32, tag="mask_u")
                nc.vector.tensor_scalar(
                    u_s[:], d2_p[:], float(CUTOFF * CUTOFF), BIG_D2,
                    ALU.is_ge, ALU.mult,
                )
                d2m_s = wpool.tile([P, N_ATOMS], FP32, tag="d2m")
                nc.vector.tensor_tensor(d2m_s[:], u_s[:], d2_p[:], ALU.add)
                dist_s = wpool.tile([P, N_ATOMS], FP32, tag=f"dist{t}")
                nc.scalar.activation(
                    dist_s[:], d2m_s[:], AF.Sqrt,
                    bias=rbfb_s[:, N_RBF:N_RBF + 1],
                )
                dist_tiles.append(dist_s)

            # ---- fused RBF + neighbor-sum: one ACT op per (tile, center) ----
            for t in range(N_TILES):
                feat_s = mpool.tile([P, N_RBF], FP32, tag=f"feat{t}")
                feat_tiles.append(feat_s)
                for k in range(N_RBF):
                    g_s = wpool.tile([P, N_ATOMS], FP32, tag="gscratch")
                    nc.scalar.activation(
                        g_s[:],
                        dist_tiles[t][:],
                        AF.Derivative_Erf,
                        bias=rbfb_s[:, k:k + 1],
                        scale=float(sqrtg),
                        accum_out=feat_s[:, k:k + 1],
                    )


            # ---- tiny MLP on PE (both atom tiles fused: N = 256) ----
            featT_p = psmall.tile([N_RBF, N_TILES * P], FP32, tag="featT")
            for t in range(N_TILES):
                nc.tensor.transpose(
                    featT_p[:, t * P:(t + 1) * P], feat_tiles[t][:], ident_s[:]
                )
            featT_s = mpool.tile([N_RBF, N_TILES * P], FP32, tag="featT_s")
            nc.vector.tensor_copy(featT_s[:], featT_p[:])
            h_p = psmall.tile([N_HIDDEN, N_TILES * P], FP32, tag="h")
            nc.tensor.matmul(h_p[:], w1f_s[:], featT_s[:], start=True, stop=True)
            hT_s = mpool.tile([N_HIDDEN, N_TILES * P], FP32, tag="hT")
            nc.scalar.activation(
                hT_s[:], h_p[:], AF.Silu, bias=b1p_s[:, 0:1], scale=1.0
            )
            e_p = psmall.tile([1, N_TILES * P], FP32, tag="e")
            nc.tensor.matmul(e_p[:], w2_s[:], hT_s[:], start=True, stop=True)
            e_s = mpool.tile([1, N_TILES * P], FP32, tag="e_s")
            nc.vector.tensor_copy(e_s[:], e_p[:])
            nc.sync.dma_start(eout_d.rearrange("t p -> (t p)")[None, :], e_s[:])


def _get_program(reps=1):
    key = ("nc", reps)
    if key not in _CACHE:
        _CACHE[key] = _build_program(reps)
    return _CACHE[key]


def _host_prep(positions, charge_state, emb_table, W1, b1, W2, b2):
    pos = np.ascontiguousarray(np.asarray(positions, dtype=np.float32))
    n = pos.shape[0]
    assert n == N_ATOMS

    sq = (pos.astype(np.float64) ** 2).sum(-1).astype(np.float32)
    ones = np.ones(n, dtype=np.float32)
    # rhs rows: [-2px, -2py, -2pz, 1, sq]; lhsT rows: [px, py, pz, sq, 1]
    rhs = np.stack([-2.0 * pos[:, 0], -2.0 * pos[:, 1], -2.0 * pos[:, 2], ones, sq])
    rhs = np.ascontiguousarray(rhs.astype(np.float32))
    lhsT_all = np.stack([pos[:, 0], pos[:, 1], pos[:, 2], sq, ones])
    lhsT_all = np.ascontiguousarray(lhsT_all.astype(np.float32))

    W1 = np.asarray(W1, dtype=np.float32)
    b1 = np.asarray(b1, dtype=np.float32)
    W2 = np.asarray(W2, dtype=np.float32)
    emb_table = np.asarray(emb_table, dtype=np.float32)
    cs_idx = 0 if int(charge_state) < 0 else 1
    emb = emb_table[cs_idx].astype(np.float64)

    # Fold: the 2/sqrt(pi) of Derivative_Erf into W1's RBF rows, and the
    # constant embedding contribution into the bias.
    w1f = (W1[:N_RBF].astype(np.float64) * (np.sqrt(np.pi) / 2.0)).astype(np.float32)
    b1p = (b1.astype(np.float64) + emb @ W1[N_RBF:].astype(np.float64)).astype(
        np.float32
    )

    ident = np.eye(P, dtype=np.float32)
    bident = (BIG_D2 * np.eye(P)).astype(np.float32)
    centers, gamma, sqrtg = _rbf_constants()
    kbias = (-(np.float64(sqrtg) * centers.astype(np.float64))).astype(np.float32)
    rbfb = np.zeros((P, N_RBF + 1), np.float32)
    rbfb[:, :N_RBF] = kbias[None, :]
    rbfb[:, N_RBF] = SQRT_BIAS

    in_maps = []
    for r in range(N_CORES):
        # rotate columns so this core's own atoms sit at columns [0, 256)
        rhs_rot = np.ascontiguousarray(np.roll(rhs, -r * ATOMS_PER_CORE, axis=1))
        in_maps.append(
            {
                "lhsT": np.ascontiguousarray(
                    lhsT_all[:, r * ATOMS_PER_CORE:(r + 1) * ATOMS_PER_CORE]
                ),
                "rhs": rhs_rot,
                "w1f": np.ascontiguousarray(w1f),
                "w2": np.ascontiguousarray(W2.reshape(N_HIDDEN, 1)),
                "b1p": np.ascontiguousarray(b1p.reshape(N_HIDDEN, 1)),
                "ident": ident,
                "bident": bident,
                "rbfb": rbfb,
            }
        )
    return in_maps


def _run(in_maps, trace=False, reps=1):
    from concourse.bass_utils import run_bass_kernel_spmd

    nc = _get_program(reps)
    return run_bass_kernel_spmd(nc, in_maps, list(range(N_CORES)), trace=trace)


def kernel(positions, charge_state, emb_table, W1, b1, W2, b2):
    in_maps = _host_prep(positions, charge_state, emb_table, W1, b1, W2, b2)
    res = _run(in_maps, trace=False)

    b2v = float(np.asarray(b2, dtype=np.float64).reshape(-1)[0])
    total = 0.0
    for r in range(N_CORES):
        e = np.asarray(res.results[r]["eout"], dtype=np.float64)
        total += e.sum()
    total += N_ATOMS * b2v
    return np.float32(total)


def profile_hw(inputs):
    """Run once with NTFF tracing; returns exec_time_ns (or None)."""
    in_maps = _host_prep(**inputs)
    res = _run(in_maps, trace=True)
    return res.exec_time_ns


def bench_hw(inputs, r_lo=256, r_hi=2048, rounds=3, n_meas=3):
    """Marginal per-iteration HW time via an on-device For_i repetition loop.

    Wall-clocks programs that run the kernel body r_lo and r_hi times inside
    one launch; the difference cancels dispatch/jit overhead. The shared
    device is noisy, so take the median marginal over interleaved rounds.
    Returns ns.
    """
    import time

    in_maps = _host_prep(**inputs)

    def t_once(reps):
        t0 = time.time()
        _run(in_maps, reps=reps)
        return time.time() - t0

    t_once(r_lo)  # warm compile + dispatch caches
    t_once(r_hi)
    marginals = []
    for _ in range(rounds):
        lo = min(t_once(r_lo) for _ in range(n_meas))
        hi = min(t_once(r_hi) for _ in range(n_meas))
        marginals.append((hi - lo) / (r_hi - r_lo))
    marginals.sort()
    return marginals[len(marginals) // 2] * 1e9


# revision 41
# speedup vs baseline: 1.1724x; 1.1724x over previous
"""Trainium2 Bass kernel for nn_Ag3ChargeStateModel (GNN message passing).

Strategy (8 NeuronCores, SPMD):
  - Shard atoms across cores: core r owns atoms [r*256, (r+1)*256), processed
    as 2 partition-tiles of 128 atoms. Positions replicated to every core.
  - d2[i,j] = |pi|^2 + |pj|^2 - 2 pi.pj via one PE matmul with a rank-5
    contraction: lhsT = [px,py,pz,sq,1] (K=5), rhs = [-2px,-2py,-2pz,1,sq].
    Per-core rhs columns are rotated so the core's own atoms sit at columns
    [0, 256); a BIG*I accumulate-matmul then spikes the self-pair diagonal.
  - Cutoff mask (d2 < cutoff^2) folded into the distances on DVE: masked-out
    pairs get d2 += 1e8 so dist ~ 1e4 and every RBF underflows to exactly 0.
  - RBF expansion + neighbor sum fused into ONE scalar-engine op per center:
    Derivative_Erf(sqrt(g)*d - sqrt(g)*c_k) = (2/sqrt(pi))*exp(-g(d-c_k)^2),
    with accum_out performing the row (neighbor) reduction. The 2/sqrt(pi)
    factor is folded into W1.
  - MLP on PE: feat^T via PE transpose, h^T = Silu(W1f^T @ feat^T + b1'),
    e^T = W2^T @ h^T. The charge-state embedding contribution is constant
    across atoms, so emb @ W1[16:] is folded into the bias b1'.
  - Per-atom energies DMA'd out; host sums the 8 partial results (psum).
"""

import numpy as np

N_ATOMS = 2048
N_CORES = 8
ATOMS_PER_CORE = N_ATOMS // N_CORES  # 256
P = 128                              # partition tile
N_TILES = ATOMS_PER_CORE // P        # 2
N_RBF = 16
N_HIDDEN = 32
CUTOFF = 5.0
BIG_D2 = 1.0e8                       # masked pairs: dist=1e4 -> RBF arg ~3e4 -> 0
SQRT_BIAS = 4.0e-5                   # keeps the sqrt input positive under f32 cancellation noise

_CACHE = {}


def _rbf_constants():
    centers = np.linspace(0.0, np.float32(CUTOFF), N_RBF, dtype=np.float32)
    width = centers[1] - centers[0]
    gamma = np.float32(1.0) / (width * width)
    sqrtg = np.float32(np.sqrt(np.float64(gamma)))
    return centers, gamma, sqrtg


def _build_program(reps=1):
    from concourse import mybir, bacc
    import concourse.tile as tile

    AF = mybir.ActivationFunctionType
    ALU = mybir.AluOpType
    FP32 = mybir.dt.float32

    centers, gamma, sqrtg = _rbf_constants()

    nc = bacc.Bacc("TRN2", target_bir_lowering=False, debug=False)

    lhsT_d = nc.dram_tensor("lhsT", [5, ATOMS_PER_CORE], FP32, kind="ExternalInput").ap()
    rhs_d = nc.dram_tensor("rhs", [5, N_ATOMS], FP32, kind="ExternalInput").ap()
    w1f_d = nc.dram_tensor("w1f", [N_RBF, N_HIDDEN], FP32, kind="ExternalInput").ap()
    w2_d = nc.dram_tensor("w2", [N_HIDDEN, 1], FP32, kind="ExternalInput").ap()
    b1p_d = nc.dram_tensor("b1p", [N_HIDDEN, 1], FP32, kind="ExternalInput").ap()
    ident_d = nc.dram_tensor("ident", [P, P], FP32, kind="ExternalInput").ap()
    bident_d = nc.dram_tensor("bident", [P, P], FP32, kind="ExternalInput").ap()
    rbfb_d = nc.dram_tensor("rbfb", [P, N_RBF + 1], FP32, kind="ExternalInput").ap()
    eout_d = nc.dram_tensor("eout", [N_TILES, P], FP32, kind="ExternalOutput").ap()

    with tile.TileContext(nc) as tc:
        with (
            tc.tile_pool(name="const", bufs=1) as cpool,
            tc.tile_pool(name="work", bufs=2) as wpool,
            tc.tile_pool(name="mlp", bufs=1) as mpool,
            tc.tile_pool(name="psum_big", bufs=1, space="PSUM") as pbig,
            tc.tile_pool(name="psum_small", bufs=1, space="PSUM") as psmall,
        ):
            # ---- constant loads ----
            rhs_s = cpool.tile([5, N_ATOMS], FP32, tag="rhs")
            nc.sync.dma_start(rhs_s[:], rhs_d[:])
            lhsT_s = cpool.tile([5, ATOMS_PER_CORE], FP32, tag="lhsT")
            nc.sync.dma_start(lhsT_s[:], lhsT_d[:])
            w1f_s = cpool.tile([N_RBF, N_HIDDEN], FP32, tag="w1f")
            nc.sync.dma_start(w1f_s[:], w1f_d[:])
            w2_s = cpool.tile([N_HIDDEN, 1], FP32, tag="w2")
            nc.sync.dma_start(w2_s[:], w2_d[:])
            b1p_s = cpool.tile([N_HIDDEN, 1], FP32, tag="b1p")
            nc.sync.dma_start(b1p_s[:], b1p_d[:])
            ident_s = cpool.tile([P, P], FP32, tag="ident")
            nc.sync.dma_start(ident_s[:], ident_d[:])
            rbfb_s = cpool.tile([P, N_RBF + 1], FP32, tag="rbfb")
            nc.sync.dma_start(rbfb_s[:], rbfb_d[:])
            bident_s = cpool.tile([P, P], FP32, tag="bident")
            nc.sync.dma_start(bident_s[:], bident_d[:])

            def body():
                _emit_body(
                    nc, tc, wpool, mpool, pbig, psmall,
                    lhsT_s, rhs_s, w1f_s, w2_s, b1p_s, ident_s, rbfb_s, bident_s,
                    eout_d, sqrtg, AF, ALU, mybir, FP32,
                )

            if reps == 1:
                body()
            else:
                with tc.For_i(0, reps, 1, staggered_reset=True):
                    body()

    nc.compile()
    return nc


def _emit_body(
    nc, tc, wpool, mpool, pbig, psmall,
    lhsT_s, rhs_s, w1f_s, w2_s, b1p_s, ident_s, rbfb_s, bident_s,
    eout_d, sqrtg, AF, ALU, mybir, FP32,
):
            dist_tiles = []
            feat_tiles = []

            # ---- distances + masking (PE + DVE), per atom tile ----
            for t in range(N_TILES):
                d2_p = pbig.tile([P, N_ATOMS], FP32, tag="d2")
                for nb in range(N_ATOMS // 512):
                    nc.tensor.matmul(
                        d2_p[:, nb * 512:(nb + 1) * 512],
                        lhsT_s[:, t * P:(t + 1) * P],
                        rhs_s[:, nb * 512:(nb + 1) * 512],
                        start=True,
                        stop=(nb != 0),
                    )
                # rhs columns are rotated per core so this core's own atoms sit
                # at columns [0, 256): spike the self-pair diagonal with BIG*I
                # so it lands beyond the cutoff (exact diagonal exclusion).
                nc.tensor.matmul(
                    d2_p[:, t * P:(t + 1) * P],
                    bident_s[:],
                    ident_s[:],
                    start=False,
                    stop=True,
                )
                # cutoff mask: add BIG where d2 >= cutoff^2 (diagonal already
                # spiked beyond the cutoff by the identity matmul above).
                u_s = wpool.tile([P, N_ATOMS], FP32, tag="mask_u")
                nc.vector.tensor_scalar(
                    u_s[:], d2_p[:], float(CUTOFF * CUTOFF), BIG_D2,
                    ALU.is_ge, ALU.mult,
                )
                d2m_s = wpool.tile([P, N_ATOMS], FP32, tag="d2m")
                nc.vector.tensor_tensor(d2m_s[:], u_s[:], d2_p[:], ALU.add)
                dist_s = wpool.tile([P, N_ATOMS], FP32, tag=f"dist{t}")
                nc.scalar.activation(
                    dist_s[:], d2m_s[:], AF.Sqrt,
                    bias=rbfb_s[:, N_RBF:N_RBF + 1],
                )
                dist_tiles.append(dist_s)

            # ---- fused RBF + neighbor-sum: one ACT op per (tile, center) ----
            for t in range(N_TILES):
                feat_s = mpool.tile([P, N_RBF], FP32, tag=f"feat{t}")
                feat_tiles.append(feat_s)
                for k in range(N_RBF):
                    g_s = wpool.tile([P, N_ATOMS], FP32, tag="gscratch")
                    nc.scalar.activation(
                        g_s[:],
                        dist_tiles[t][:],
                        AF.Derivative_Erf,
                        bias=rbfb_s[:, k:k + 1],
                        scale=float(sqrtg),
                        accum_out=feat_s[:, k:k + 1],
                    )


            # ---- tiny MLP on PE (both atom tiles fused: N = 256) ----
            featT_p = psmall.tile([N_RBF, N_TILES * P], FP32, tag="featT")
            for t in range(N_TILES):
                nc.tensor.transpose(
                    featT_p[:, t * P:(t + 1) * P], feat_tiles[t][:], ident_s[:]
                )
            featT_s = mpool.tile([N_RBF, N_TILES * P], FP32, tag="featT_s")
            nc.vector.tensor_copy(featT_s[:], featT_p[:])
            h_p = psmall.tile([N_HIDDEN, N_TILES * P], FP32, tag="h")
            nc.tensor.matmul(h_p[:], w1f_s[:], featT_s[:], start=True, stop=True)
            hT_s = mpool.tile([N_HIDDEN, N_TILES * P], FP32, tag="hT")
            nc.scalar.activation(
                hT_s[:], h_p[:], AF.Silu, bias=b1p_s[:, 0:1], scale=1.0
            )
            e_p = psmall.tile([1, N_TILES * P], FP32, tag="e")
            nc.tensor.matmul(e_p[:], w2_s[:], hT_s[:], start=True, stop=True)
            e_s = mpool.tile([1, N_TILES * P], FP32, tag="e_s")
            nc.vector.tensor_copy(e_s[:], e_p[:])
            nc.sync.dma_start(eout_d.rearrange("t p -> (t p)")[None, :], e_s[:])


def _get_program(reps=1):
    key = ("nc", reps)
    if key not in _CACHE:
        _CACHE[key] = _build_program(reps)
    return _CACHE[key]


def _host_prep(positions, charge_state, emb_table, W1, b1, W2, b2):
    pos = np.ascontiguousarray(np.asarray(positions, dtype=np.float32))
    n = pos.shape[0]
    assert n == N_ATOMS

    sq = (pos.astype(np.float64) ** 2).sum(-1).astype(np.float32)
    ones = np.ones(n, dtype=np.float32)
    # rhs rows: [-2px, -2py, -2pz, 1, sq]; lhsT rows: [px, py, pz, sq, 1]
    rhs = np.stack([-2.0 * pos[:, 0], -2.0 * pos[:, 1], -2.0 * pos[:, 2], ones, sq])
    rhs = np.ascontiguousarray(rhs.astype(np.float32))
    lhsT_all = np.stack([pos[:, 0], pos[:, 1], pos[:, 2], sq, ones])
    lhsT_all = np.ascontiguousarray(lhsT_all.astype(np.float32))

    W1 = np.asarray(W1, dtype=np.float32)
    b1 = np.asarray(b1, dtype=np.float32)
    W2 = np.asarray(W2, dtype=np.float32)
    emb_table = np.asarray(emb_table, dtype=np.float32)
    cs_idx = 0 if int(charge_state) < 0 else 1
    emb = emb_table[cs_idx].astype(np.float64)

    # Fold: the 2/sqrt(pi) of Derivative_Erf into W1's RBF rows, and the
    # constant embedding contribution into the bias.
    w1f = (W1[:N_RBF].astype(np.float64) * (np.sqrt(np.pi) / 2.0)).astype(np.float32)
    b1p = (b1.astype(np.float64) + emb @ W1[N_RBF:].astype(np.float64)).astype(
        np.float32
    )

    ident = np.eye(P, dtype=np.float32)
    bident = (BIG_D2 * np.eye(P)).astype(np.float32)
    centers, gamma, sqrtg = _rbf_constants()
    kbias = (-(np.float64(sqrtg) * centers.astype(np.float64))).astype(np.float32)
    rbfb = np.zeros((P, N_RBF + 1), np.float32)
    rbfb[:, :N_RBF] = kbias[None, :]
    rbfb[:, N_RBF] = SQRT_BIAS

    in_maps = []
    for r in range(N_CORES):
        # rotate columns so this core's own atoms sit at columns [0, 256)
        rhs_rot = np.ascontiguousarray(np.roll(rhs, -r * ATOMS_PER_CORE, axis=1))
        in_maps.append(
            {
                "lhsT": np.ascontiguousarray(
                    lhsT_all[:, r * ATOMS_PER_CORE:(r + 1) * ATOMS_PER_CORE]
                ),
                "rhs": rhs_rot,
                "w1f": np.ascontiguousarray(w1f),
                "w2": np.ascontiguousarray(W2.reshape(N_HIDDEN, 1)),
                "b1p": np.ascontiguousarray(b1p.reshape(N_HIDDEN, 1)),
                "ident": ident,
                "bident": bident,
                "rbfb": rbfb,
            }
        )
    return in_maps


def _run(in_maps, trace=False, reps=1):
    from concourse.bass_utils import run_bass_kernel_spmd

    nc = _get_program(reps)
    return run_bass_kernel_spmd(nc, in_maps, list(range(N_CORES)), trace=trace)


def kernel(positions, charge_state, emb_table, W1, b1, W2, b2):
    in_maps = _host_prep(positions, charge_state, emb_table, W1, b1, W2, b2)
    res = _run(in_maps, trace=False)

    b2v = float(np.asarray(b2, dtype=np.float64).reshape(-1)[0])
    total = 0.0
    for r in range(N_CORES):
        e = np.asarray(res.results[r]["eout"], dtype=np.float64)
        total += e.sum()
    total += N_ATOMS * b2v
    return np.float32(total)


def profile_hw(inputs):
    """Run once with NTFF tracing; returns exec_time_ns (or None)."""
    in_maps = _host_prep(**inputs)
    res = _run(in_maps, trace=True)
    return res.exec_time_ns


def bench_hw(inputs, r_lo=256, r_hi=2048, rounds=3, n_meas=3):
    """Marginal per-iteration HW time via an on-device For_i repetition loop.

    Wall-clocks programs that run the kernel body r_lo and r_hi times inside
    one launch; the difference cancels dispatch/jit overhead. The shared
    device is noisy, so take the median marginal over interleaved rounds.
    Returns ns.
    """
    import time

    in_maps = _host_prep(**inputs)

    def t_once(reps):
        t0 = time.time()
        _run(in_maps, reps=reps)
        return time.time() - t0

    t_once(r_lo)  # warm compile + dispatch caches
    t_once(r_hi)
    marginals = []
    for _ in range(rounds):
        lo = min(t_once(r_lo) for _ in range(n_meas))
        hi = min(t_once(r_hi) for _ in range(n_meas))
        marginals.append((hi - lo) / (r_hi - r_lo))
    marginals.sort()
    return marginals[len(marginals) // 2] * 1e9


# revision 56
# speedup vs baseline: 1.2348x; 1.0532x over previous
"""Trainium2 Bass kernel for nn_Ag3ChargeStateModel (GNN message passing).

Strategy (8 NeuronCores, SPMD):
  - Shard atoms across cores: core r owns atoms [r*256, (r+1)*256), processed
    as 2 partition-tiles of 128 atoms. Positions replicated to every core.
  - d2[i,j] = |pi|^2 + |pj|^2 - 2 pi.pj via one PE matmul with a rank-5
    contraction: lhsT = [px,py,pz,sq,1] (K=5), rhs = [-2px,-2py,-2pz,1,sq].
    Per-core rhs columns are rotated so the core's own atoms sit at columns
    [0, 256); a BIG*I accumulate-matmul then spikes the self-pair diagonal.
  - Cutoff mask (d2 < cutoff^2) folded into the distances on DVE: masked-out
    pairs get d2 += 1e8 so dist ~ 1e4 and every RBF underflows to exactly 0.
  - RBF expansion + neighbor sum fused into ONE scalar-engine op per center:
    Derivative_Erf(sqrt(g)*d - sqrt(g)*c_k) = (2/sqrt(pi))*exp(-g(d-c_k)^2),
    with accum_out performing the row (neighbor) reduction. The 2/sqrt(pi)
    factor is folded into W1.
  - MLP on PE: feat^T via PE transpose, h^T = Silu(W1f^T @ feat^T + b1'),
    e^T = W2^T @ h^T. The charge-state embedding contribution is constant
    across atoms, so emb @ W1[16:] is folded into the bias b1'.
  - Per-atom energies DMA'd out; host sums the 8 partial results (psum).
"""

import numpy as np

N_ATOMS = 2048
N_CORES = 8
ATOMS_PER_CORE = N_ATOMS // N_CORES  # 256
P = 128                              # partition tile
N_TILES = ATOMS_PER_CORE // P        # 2
N_RBF = 16
N_HIDDEN = 32
CUTOFF = 5.0
BIG_D2 = 1.0e8                       # masked pairs: dist=1e4 -> RBF arg ~3e4 -> 0
SQRT_BIAS = 4.0e-5                   # keeps the sqrt input positive under f32 cancellation noise

_CACHE = {}


def _rbf_constants():
    centers = np.linspace(0.0, np.float32(CUTOFF), N_RBF, dtype=np.float32)
    width = centers[1] - centers[0]
    gamma = np.float32(1.0) / (width * width)
    sqrtg = np.float32(np.sqrt(np.float64(gamma)))
    return centers, gamma, sqrtg


def _build_program(reps=1, w=N_ATOMS):
    from concourse import mybir, bacc
    import concourse.tile as tile

    AF = mybir.ActivationFunctionType
    ALU = mybir.AluOpType
    FP32 = mybir.dt.float32

    centers, gamma, sqrtg = _rbf_constants()

    nc = bacc.Bacc("TRN2", target_bir_lowering=False, debug=False)

    lhsT_d = nc.dram_tensor("lhsT", [5, ATOMS_PER_CORE], FP32, kind="ExternalInput").ap()
    rhs_d = nc.dram_tensor("rhs", [5, w], FP32, kind="ExternalInput").ap()
    w1f_d = nc.dram_tensor("w1f", [N_RBF, N_HIDDEN], FP32, kind="ExternalInput").ap()
    w2_d = nc.dram_tensor("w2", [N_HIDDEN, 1], FP32, kind="ExternalInput").ap()
    b1p_d = nc.dram_tensor("b1p", [N_HIDDEN, 1], FP32, kind="ExternalInput").ap()
    ident_d = nc.dram_tensor("ident", [P, P], FP32, kind="ExternalInput").ap()
    bident_d = nc.dram_tensor("bident", [P, P], FP32, kind="ExternalInput").ap()
    rbfb_d = nc.dram_tensor("rbfb", [P, N_RBF + 1], FP32, kind="ExternalInput").ap()
    eout_d = nc.dram_tensor("eout", [N_TILES, P], FP32, kind="ExternalOutput").ap()

    with tile.TileContext(nc) as tc:
        with (
            tc.tile_pool(name="const", bufs=1) as cpool,
            tc.tile_pool(name="work", bufs=2) as wpool,
            tc.tile_pool(name="mlp", bufs=1) as mpool,
            tc.tile_pool(name="psum_big", bufs=1, space="PSUM") as pbig,
            tc.tile_pool(name="psum_small", bufs=1, space="PSUM") as psmall,
        ):
            # ---- constant loads ----
            rhs_s = cpool.tile([5, w], FP32, tag="rhs")
            nc.sync.dma_start(rhs_s[:], rhs_d[:])
            lhsT_s = cpool.tile([5, ATOMS_PER_CORE], FP32, tag="lhsT")
            nc.sync.dma_start(lhsT_s[:], lhsT_d[:])
            w1f_s = cpool.tile([N_RBF, N_HIDDEN], FP32, tag="w1f")
            nc.sync.dma_start(w1f_s[:], w1f_d[:])
            w2_s = cpool.tile([N_HIDDEN, 1], FP32, tag="w2")
            nc.sync.dma_start(w2_s[:], w2_d[:])
            b1p_s = cpool.tile([N_HIDDEN, 1], FP32, tag="b1p")
            nc.sync.dma_start(b1p_s[:], b1p_d[:])
            ident_s = cpool.tile([P, P], FP32, tag="ident")
            nc.sync.dma_start(ident_s[:], ident_d[:])
            rbfb_s = cpool.tile([P, N_RBF + 1], FP32, tag="rbfb")
            nc.sync.dma_start(rbfb_s[:], rbfb_d[:])
            bident_s = cpool.tile([P, P], FP32, tag="bident")
            nc.sync.dma_start(bident_s[:], bident_d[:])

            def body():
                _emit_body(
                    nc, tc, wpool, mpool, pbig, psmall,
                    lhsT_s, rhs_s, w1f_s, w2_s, b1p_s, ident_s, rbfb_s, bident_s,
                    eout_d, sqrtg, AF, ALU, mybir, FP32, w,
                )

            if reps == 1:
                body()
            else:
                with tc.For_i(0, reps, 1, staggered_reset=True):
                    body()

    nc.compile()
    return nc


def _emit_body(
    nc, tc, wpool, mpool, pbig, psmall,
    lhsT_s, rhs_s, w1f_s, w2_s, b1p_s, ident_s, rbfb_s, bident_s,
    eout_d, sqrtg, AF, ALU, mybir, FP32, w,
):
            dist_tiles = []
            feat_tiles = []

            # Preload the sqrt activation-table set during the DMA/PE head:
            # a dependency-free dummy op triggers the ~2.7us ACT_TABLE_LOAD
            # while the engine would otherwise sit idle.
            warm_s = wpool.tile([1, 1], FP32, tag="warm")
            nc.scalar.activation(warm_s[:], rbfb_s[0:1, N_RBF:N_RBF + 1], AF.Sqrt)

            # ---- distances + masking (PE + DVE), per atom tile ----
            for t in range(N_TILES):
                d2_p = pbig.tile([P, w], FP32, tag="d2")
                for nb, c0 in enumerate(range(0, w, 512)):
                    c1 = min(c0 + 512, w)
                    nc.tensor.matmul(
                        d2_p[:, c0:c1],
                        lhsT_s[:, t * P:(t + 1) * P],
                        rhs_s[:, c0:c1],
                        start=True,
                        stop=(nb != 0),
                    )
                # rhs columns are rotated per core so this core's own atoms sit
                # at columns [0, 256): spike the self-pair diagonal with BIG*I
                # so it lands beyond the cutoff (exact diagonal exclusion).
                nc.tensor.matmul(
                    d2_p[:, t * P:(t + 1) * P],
                    bident_s[:],
                    ident_s[:],
                    start=False,
                    stop=True,
                )
                # cutoff mask: add BIG where d2 >= cutoff^2 (diagonal already
                # spiked beyond the cutoff by the identity matmul above).
                u_s = wpool.tile([P, w], FP32, tag="mask_u")
                nc.vector.tensor_scalar(
                    u_s[:], d2_p[:], float(CUTOFF * CUTOFF), BIG_D2,
                    ALU.is_ge, ALU.mult,
                )
                d2m_s = wpool.tile([P, w], FP32, tag="d2m")
                nc.vector.tensor_tensor(d2m_s[:], u_s[:], d2_p[:], ALU.add)
                dist_s = wpool.tile([P, w], FP32, tag=f"dist{t}")
                nc.scalar.activation(
                    dist_s[:], d2m_s[:], AF.Sqrt,
                    bias=rbfb_s[:, N_RBF:N_RBF + 1],
                )
                dist_tiles.append(dist_s)

            # ---- fused RBF + neighbor-sum: one ACT op per (tile, center) ----
            for t in range(N_TILES):
                feat_s = mpool.tile([P, N_RBF], FP32, tag=f"feat{t}")
                feat_tiles.append(feat_s)
                for k in range(N_RBF):
                    g_s = wpool.tile([P, w], FP32, tag="gscratch")
                    nc.scalar.activation(
                        g_s[:],
                        dist_tiles[t][:],
                        AF.Derivative_Erf,
                        bias=rbfb_s[:, k:k + 1],
                        scale=float(sqrtg),
                        accum_out=feat_s[:, k:k + 1],
                    )


            # ---- tiny MLP on PE (both atom tiles fused: N = 256) ----
            featT_p = psmall.tile([N_RBF, N_TILES * P], FP32, tag="featT")
            for t in range(N_TILES):
                nc.tensor.transpose(
                    featT_p[:, t * P:(t + 1) * P], feat_tiles[t][:], ident_s[:]
                )
            featT_s = mpool.tile([N_RBF, N_TILES * P], FP32, tag="featT_s")
            nc.vector.tensor_copy(featT_s[:], featT_p[:])
            h_p = psmall.tile([N_HIDDEN, N_TILES * P], FP32, tag="h")
            nc.tensor.matmul(h_p[:], w1f_s[:], featT_s[:], start=True, stop=True)
            hT_s = mpool.tile([N_HIDDEN, N_TILES * P], FP32, tag="hT")
            nc.scalar.activation(
                hT_s[:], h_p[:], AF.Silu, bias=b1p_s[:, 0:1], scale=1.0
            )
            e_p = psmall.tile([1, N_TILES * P], FP32, tag="e")
            nc.tensor.matmul(e_p[:], w2_s[:], hT_s[:], start=True, stop=True)
            e_s = mpool.tile([1, N_TILES * P], FP32, tag="e_s")
            nc.vector.tensor_copy(e_s[:], e_p[:])
            nc.sync.dma_start(eout_d.rearrange("t p -> (t p)")[None, :], e_s[:])


def _get_program(reps=1, w=N_ATOMS):
    key = ("nc", reps, w)
    if key not in _CACHE:
        _CACHE[key] = _build_program(reps, w)
    return _CACHE[key]


def _choose_order(pos):
    """Sort atoms along the projection direction minimizing the widest
    per-core neighbor window (columns outside slab+-cutoff can never be
    neighbors of that core's atoms)."""
    dirs = [np.eye(3)[i] for i in range(3)]
    rng = np.random.RandomState(7)
    for _ in range(13):
        v = rng.randn(3)
        dirs.append(v / np.linalg.norm(v))
    best = None
    for v in dirs:
        proj = pos.astype(np.float64) @ v
        order = np.argsort(proj, kind="stable")
        ps = proj[order]
        wmax = 0
        for r in range(N_CORES):
            slab = ps[r * ATOMS_PER_CORE:(r + 1) * ATOMS_PER_CORE]
            lo, hi = slab[0] - CUTOFF, slab[-1] + CUTOFF
            wmax = max(wmax, int(((ps > lo) & (ps < hi)).sum()))
        if best is None or wmax < best[0]:
            best = (wmax, order, ps)
    return best


def _host_prep(positions, charge_state, emb_table, W1, b1, W2, b2):
    pos_in = np.ascontiguousarray(np.asarray(positions, dtype=np.float32))
    n = pos_in.shape[0]
    assert n == N_ATOMS

    wmax, order, ps = _choose_order(pos_in)
    pos = pos_in[order]
    w = min(N_ATOMS, max(512, -(-wmax // 128) * 128))

    sq = (pos.astype(np.float64) ** 2).sum(-1).astype(np.float32)
    ones = np.ones(n, dtype=np.float32)
    # rhs rows: [-2px, -2py, -2pz, 1, sq]; lhsT rows: [px, py, pz, sq, 1]
    rhs = np.stack([-2.0 * pos[:, 0], -2.0 * pos[:, 1], -2.0 * pos[:, 2], ones, sq])
    rhs = np.ascontiguousarray(rhs.astype(np.float32))
    lhsT_all = np.stack([pos[:, 0], pos[:, 1], pos[:, 2], sq, ones])
    lhsT_all = np.ascontiguousarray(lhsT_all.astype(np.float32))

    W1 = np.asarray(W1, dtype=np.float32)
    b1 = np.asarray(b1, dtype=np.float32)
    W2 = np.asarray(W2, dtype=np.float32)
    emb_table = np.asarray(emb_table, dtype=np.float32)
    cs_idx = 0 if int(charge_state) < 0 else 1
    emb = emb_table[cs_idx].astype(np.float64)

    # Fold: the 2/sqrt(pi) of Derivative_Erf into W1's RBF rows, and the
    # constant embedding contribution into the bias.
    w1f = (W1[:N_RBF].astype(np.float64) * (np.sqrt(np.pi) / 2.0)).astype(np.float32)
    b1p = (b1.astype(np.float64) + emb @ W1[N_RBF:].astype(np.float64)).astype(
        np.float32
    )

    ident = np.eye(P, dtype=np.float32)
    bident = (BIG_D2 * np.eye(P)).astype(np.float32)
    centers, gamma, sqrtg = _rbf_constants()
    kbias = (-(np.float64(sqrtg) * centers.astype(np.float64))).astype(np.float32)
    rbfb = np.zeros((P, N_RBF + 1), np.float32)
    rbfb[:, :N_RBF] = kbias[None, :]
    rbfb[:, N_RBF] = SQRT_BIAS

    in_maps = []
    for r in range(N_CORES):
        # window: sorted atoms within slab +- cutoff; own atoms first so the
        # diagonal spike lands at columns [0, 256); pad to w with far dummies
        a0 = r * ATOMS_PER_CORE
        slab_lo, slab_hi = ps[a0] - CUTOFF, ps[a0 + ATOMS_PER_CORE - 1] + CUTOFF
        win = np.nonzero((ps > slab_lo) & (ps < slab_hi))[0]
        others = win[(win < a0) | (win >= a0 + ATOMS_PER_CORE)]
        cols = np.concatenate([np.arange(a0, a0 + ATOMS_PER_CORE), others])
        assert len(cols) <= w
        rhs_r = np.empty((5, w), np.float32)
        rhs_r[:, :len(cols)] = rhs[:, cols]
        if len(cols) < w:
            rhs_r[:, len(cols):] = np.array(
                [[0.0], [0.0], [0.0], [1.0], [BIG_D2]], np.float32
            )
        in_maps.append(
            {
                "lhsT": np.ascontiguousarray(
                    lhsT_all[:, a0:a0 + ATOMS_PER_CORE]
                ),
                "rhs": np.ascontiguousarray(rhs_r),
                "w1f": np.ascontiguousarray(w1f),
                "w2": np.ascontiguousarray(W2.reshape(N_HIDDEN, 1)),
                "b1p": np.ascontiguousarray(b1p.reshape(N_HIDDEN, 1)),
                "ident": ident,
                "bident": bident,
                "rbfb": rbfb,
            }
        )
    return in_maps, w


def _run(in_maps, trace=False, reps=1, w=N_ATOMS):
    from concourse.bass_utils import run_bass_kernel_spmd

    nc = _get_program(reps, w)
    return run_bass_kernel_spmd(nc, in_maps, list(range(N_CORES)), trace=trace)


def kernel(positions, charge_state, emb_table, W1, b1, W2, b2):
    in_maps, w = _host_prep(positions, charge_state, emb_table, W1, b1, W2, b2)
    res = _run(in_maps, trace=False, w=w)

    b2v = float(np.asarray(b2, dtype=np.float64).reshape(-1)[0])
    total = 0.0
    for r in range(N_CORES):
        e = np.asarray(res.results[r]["eout"], dtype=np.float64)
        total += e.sum()
    total += N_ATOMS * b2v
    return np.float32(total)


def profile_hw(inputs):
    """Run once with NTFF tracing; returns exec_time_ns (or None)."""
    in_maps, w = _host_prep(**inputs)
    res = _run(in_maps, trace=True, w=w)
    return res.exec_time_ns


def bench_hw(inputs, r_lo=256, r_hi=2048, rounds=3, n_meas=3):
    """Marginal per-iteration HW time via an on-device For_i repetition loop.

    Wall-clocks programs that run the kernel body r_lo and r_hi times inside
    one launch; the difference cancels dispatch/jit overhead. The shared
    device is noisy, so take the median marginal over interleaved rounds.
    Returns ns.
    """
    import time

    in_maps, w = _host_prep(**inputs)

    def t_once(reps):
        t0 = time.time()
        _run(in_maps, reps=reps, w=w)
        return time.time() - t0

    t_once(r_lo)  # warm compile + dispatch caches
    t_once(r_hi)
    marginals = []
    for _ in range(rounds):
        lo = min(t_once(r_lo) for _ in range(n_meas))
        hi = min(t_once(r_hi) for _ in range(n_meas))
        marginals.append((hi - lo) / (r_hi - r_lo))
    marginals.sort()
    return marginals[len(marginals) // 2] * 1e9
